# revision 29
# baseline (speedup 1.0000x reference)
"""AttentionMIL (segment softmax-attention reduce) Trainium2 kernel, 8 NeuronCores.

Model (per reference):
    h       = relu(features @ W1.T + b1)          # [N, 256]
    a       = tanh(h @ Wa1.T + ba1)               # [N, 128]
    scores  = a @ Wa2.T + ba2                     # [N]
    attn    = segment_softmax(scores, 32 bags of 8192)
    bag_emb = segment_sum(attn * h)               # [32, 256]
    out     = bag_emb @ Wh.T + bh                 # [32, 2]

Sharding: patches split 8 ways (32768 patches = 4 whole bags per core);
weights replicated; everything device-local, no collectives.

Host prep: features are transposed and quantized (fp8-e4m3 by default) into
per-DMA-slab contiguous blocks ([P, DC, w] c-major per slab) so every slab DMA
is one contiguous per-partition descriptor run. W1/Wa1 are scaled by 16 before
fp8 quantization (their 0.02-scale values would otherwise sit in the subnormal
range); the 1/16 is folded into the activations' free scale operand. Wa2 is
replicated into a [128, 128] stationary so the score matmul lands broadcast
across all 128 partitions — making softmax + weighted reduce pure free-axis
operations. Encoder and attention matmuls run fp8 DoubleRow.

The tensor engine is the bound resource (~2.35us per 512-patch chunk of
irreducible DR matmul). The attention-side elementwise work is batched over
1024-patch groups so ScalarE/VectorE instruction overheads stay below the PE
budget; groups taper to 512 at the start (compute begins on the first 0.5MB
slab) and at the end (halves the pipeline-drain tail). Weight/const loads ride
the idle GPSIMD SWDGE ring; startup slabs alternate the two HWDGE rings; a
burst of matmuls on zeros during the initial DMA window lifts the PE HAM clock
gate to 2.4 GHz before real work arrives. PSUM: 2x2 encoder banks + 2-bank
attention pre-act + 2-bank score buffer = all 8 banks.

Softmax max-subtraction is dropped: attn = e/z is exactly shift-invariant and
|scores| <= sum|Wa2| * 1 < 3, so exp cannot overflow. ba2 likewise cancels.
"""

import sys

if "/opt/trn_rl_repo" not in sys.path:
    sys.path.insert(0, "/opt/trn_rl_repo")

from contextlib import ExitStack

import ml_dtypes
import numpy as np

from concourse import bacc, mybir, tile
from concourse.bass_utils import run_bass_kernel_spmd

N_CORES = 8
N_PATCHES = 262144
INPUT_DIM = 1024
FEAT_DIM = 256
ATTN_DIM = 128
HEAD_DIM = 2
NP_CORE = N_PATCHES // N_CORES  # 32768
BAG = 8192

P = 128
DC = INPUT_DIM // P  # 8 contraction chunks of 128
CHUNK = 512          # patches per encoder tile (one PSUM bank at fp32)
GW = 2 * CHUNK       # max attention-group width
W_SCALE = 16.0       # host pre-scale on W1/Wa1 before fp8 quantization

# DMA slab schedule: small slabs to prime the pipeline, then 2MB slabs.
SLABS = [512, 512] + [1024] * 31
def _slab_groups(k):
    w = SLABS[k]
    return [w] if w < GW else [GW] * (w // GW)

ZMAX = 10            # z-partial slots per bag (tail groups split per chunk)

BF16 = mybir.dt.bfloat16
F32 = mybir.dt.float32
FP8 = mybir.dt.float8e4
AF = mybir.ActivationFunctionType
ALU = mybir.AluOpType
AX = mybir.AxisListType
DR = mybir.MatmulPerfMode.DoubleRow

NP_F8 = ml_dtypes.float8_e4m3
NP_BF16 = ml_dtypes.bfloat16


def build_nc(np_core=NP_CORE, bag=BAG, mode="fp8"):
    n_bags = np_core // bag       # bags per core = 4
    assert sum(SLABS) == np_core and np_core % bag == 0

    fp8 = mode == "fp8"
    XDT = FP8 if fp8 else BF16

    nc = bacc.Bacc()
    xt = nc.declare_dram_parameter("xt", [P, DC * np_core], XDT, isOutput=False)
    w1t = nc.declare_dram_parameter("w1t", [P, DC, FEAT_DIM], XDT, isOutput=False)
    wa1t = nc.declare_dram_parameter("wa1t", [P, 2, ATTN_DIM], XDT, isOutput=False)
    wa2r = nc.declare_dram_parameter("wa2r", [P, P], BF16, isOutput=False)
    # packed tail consts: [wht(4) | bh2(2) | ones(n_bags) | ba1c(1)] bf16
    wpk = nc.declare_dram_parameter("wpk", [P, 7 + n_bags], BF16, isOutput=False)
    b1c = nc.declare_dram_parameter("b1c", [P, 2], F32, isOutput=False)
    ba1c = nc.declare_dram_parameter("ba1c", [P, 1], F32, isOutput=False)
    out_ext = nc.declare_dram_parameter("out", [n_bags, HEAD_DIM], F32, isOutput=True)

    inv_scale = 1.0 / W_SCALE if fp8 else 1.0

    def slab_ap(off, w):
        return xt.ap()[:, off * DC:(off + w) * DC].rearrange(
            "p (c n) -> p c n", c=DC, n=w)

    with tile.TileContext(nc) as tc, ExitStack() as ctx:
        const = ctx.enter_context(tc.tile_pool(name="const", bufs=1))
        xpool = ctx.enter_context(tc.tile_pool(name="xpool", bufs=3))
        hpool = ctx.enter_context(tc.tile_pool(name="hpool", bufs=3))
        apool = ctx.enter_context(tc.tile_pool(name="apool", bufs=2))
        epool = ctx.enter_context(tc.tile_pool(name="epool", bufs=2))
        stpool = ctx.enter_context(tc.tile_pool(name="stpool", bufs=2))
        bpool = ctx.enter_context(tc.tile_pool(name="bpool", bufs=2))
        # PSUM (8 banks): hp 2 bufs x 2 banks + ap_g 2 banks + sp_g 2 banks
        psum_h = ctx.enter_context(tc.tile_pool(name="psum_h", bufs=2, space="PSUM"))
        psum_a = ctx.enter_context(tc.tile_pool(name="psum_a", bufs=1, space="PSUM"))
        psum_s = ctx.enter_context(tc.tile_pool(name="psum_s", bufs=1, space="PSUM"))

        # PE warm-up during the initial DMA window: back-to-back matmuls on
        # zeros lift the HAM clock gate to 2.4 GHz before the first real
        # encoder matmul issues, with no idle window that would re-throttle.
        dummy = const.tile([P, CHUNK], XDT)
        warm = psum_a.tile([P, GW], F32, tag="ap")
        nc.vector.memset(dummy[:], 0.0)
        for i in range(8):
            nc.tensor.matmul(warm[:, (i % 2) * CHUNK:(i % 2 + 1) * CHUNK],
                             dummy[:, 0:P], dummy[:],
                             start=True, stop=True)

        # All DMAs ride the sync-engine HWDGE ring in dependency order —
        # w1t before slab0 (both gate the first matmul), remaining weights
        # behind slab0. A second ring would let the deep slab queue starve
        # the weight transfers (measured: wa2r +8us late, PE stall, HAM
        # re-throttle).
        # w1t + the first two slabs ride the GPSIMD SWDGE queue, which clears
        # its preamble ~2us before the HWDGE rings — first data lands sooner
        w1t_sb = const.tile([P, DC, FEAT_DIM], XDT)
        nc.gpsimd.dma_start(w1t_sb[:], w1t.ap())
        wa1t_sb = const.tile([P, 2, ATTN_DIM], XDT)
        wa2r_sb = const.tile([P, P], BF16)
        wpk_sb = const.tile([P, 7 + n_bags], BF16)
        b1c_sb = const.tile([P, 2], F32)
        ba1c_sb = const.tile([P, 1], F32)

        def load_late_consts():
            nc.sync.dma_start(wa1t_sb[:], wa1t.ap())
            nc.sync.dma_start(wa2r_sb[:], wa2r.ap())
            nc.sync.dma_start(wpk_sb[:], wpk.ap())
            if not fp8:
                nc.sync.dma_start(b1c_sb[:], b1c.ap())
                nc.sync.dma_start(ba1c_sb[:], ba1c.ap())

        tanh_bias = wpk_sb[:, 6 + n_bags:7 + n_bags] if fp8 else ba1c_sb[:]
        bagembT = const.tile([P, 2, n_bags], BF16)  # normalized bag embeddings

        def emit_stt(pend):
            # weighted partial reduce: bpp[f, fh, slot] = sum_p h.T*e, fused
            # into the multiply via the accumulate output. Deferred by one
            # group (software pipelining) so these never block the next
            # group's relu in the VectorE FIFO behind a pending exp.
            ht_, eb_, bpp_, pslot, lo, hi = pend
            st = stpool.tile([P, 2, GW], BF16, tag="st")
            nc.vector.scalar_tensor_tensor(
                st[:, 0, lo:hi], ht_[:, 0, lo:hi], 1.0, eb_[:, lo:hi],
                op0=ALU.mult, op1=ALU.mult,
                accum_out=bpp_[:, 0, pslot:pslot + 1])
            nc.vector.scalar_tensor_tensor(
                st[:, 1, lo:hi], ht_[:, 1, lo:hi], 1.0, eb_[:, lo:hi],
                op0=ALU.mult, op1=ALU.mult,
                accum_out=bpp_[:, 1, pslot:pslot + 1])

        def emit_fin(fin):
            # finish bag b: z = sum(zparts); bagembT[:, :, b] = sum/z
            b_, zp_, bpp_, ns = fin
            zs = bpool.tile([P, 1], F32, tag="zs")
            nc.vector.tensor_reduce(zs[:], zp_[:, 0:ns], axis=AX.X, op=ALU.add)
            rz = bpool.tile([P, 1], F32, tag="rz")
            nc.vector.reciprocal(rz[:], zs[:])
            s0 = bpool.tile([P, 1], F32, tag="s0")
            nc.vector.tensor_reduce(s0[:], bpp_[:, 0, 0:ns], axis=AX.X, op=ALU.add)
            s1 = bpool.tile([P, 1], F32, tag="s1")
            nc.vector.tensor_reduce(s1[:], bpp_[:, 1, 0:ns], axis=AX.X, op=ALU.add)
            nc.vector.tensor_mul(bagembT[:, 0, b_:b_ + 1], s0[:], rz[:])
            nc.vector.tensor_mul(bagembT[:, 1, b_:b_ + 1], s1[:], rz[:])

        zparts = bpp = None
        pending = []     # (ht, eb, bpp, slot, lo, hi) awaiting their stt
        pend_fin = None  # (b, zparts, bpp, nslots) awaiting bag finalize
        slot = 0
        goff = 0  # global patch offset
        off = 0
        n_groups = sum(len(_slab_groups(k)) for k in range(len(SLABS)))
        gi = 0
        for k, w in enumerate(SLABS):
            xsb = xpool.tile([P, DC, w], XDT, tag=f"x{w}")
            ring = nc.gpsimd if k < 2 else nc.sync
            ring.dma_start(xsb[:], slab_ap(off, w))
            if k == 0:
                load_late_consts()
            off += w
            coff = 0  # patch offset within slab
            for wg in _slab_groups(k):
                b = goff // bag
                if goff % bag == 0:
                    slot = 0
                    zparts = bpool.tile([P, ZMAX], F32, tag="zparts")
                    bpp = bpool.tile([P, 2, ZMAX], F32, tag="bpp")
                nchk = wg // CHUNK

                # encoder for the group's 512-patch chunks
                ht = hpool.tile([P, 2, GW], XDT, tag="ht")
                for cc in range(nchk):
                    cs = coff + cc * CHUNK
                    rhs = xsb[:, :, cs:cs + CHUNK]
                    hp = psum_h.tile([P, 2, CHUNK], F32, tag="hp")
                    for fh in range(2):
                        if fp8:
                            for d in range(DC // 2):
                                nc.tensor.matmul(
                                    hp[:, fh, :],
                                    w1t_sb[:, 2 * d:2 * d + 2, fh * P:(fh + 1) * P],
                                    rhs[:, 2 * d:2 * d + 2, :],
                                    start=(d == 0), stop=(d == DC // 2 - 1),
                                    perf_mode=DR,
                                )
                        else:
                            for d in range(DC):
                                nc.tensor.matmul(
                                    hp[:, fh, :],
                                    w1t_sb[:, d, fh * P:(fh + 1) * P],
                                    rhs[:, d, :],
                                    start=(d == 0), stop=(d == DC - 1),
                                )
                    ho = ht[:, :, cc * CHUNK:(cc + 1) * CHUNK]
                    if fp8:
                        # b1 == 0 in this model (checked in kernel()); split
                        # the two feature halves across ScalarE and VectorE
                        nc.scalar.activation(ho[:, 0, :], hp[:, 0, :], AF.Relu,
                                             bias=0.0, scale=inv_scale)
                        nc.vector.tensor_scalar(ho[:, 1, :], hp[:, 1, :],
                                                inv_scale, 0.0,
                                                op0=ALU.mult, op1=ALU.max)
                    else:
                        nc.scalar.activation(ho[:, 0, :], hp[:, 0, :], AF.Relu,
                                             bias=b1c_sb[:, 0:1])
                        nc.scalar.activation(ho[:, 1, :], hp[:, 1, :], AF.Relu,
                                             bias=b1c_sb[:, 1:2])

                # previous group's weighted reduce + any completed bag
                for pend in pending:
                    emit_stt(pend)
                pending = []
                if pend_fin is not None:
                    emit_fin(pend_fin)
                    pend_fin = None

                # attention MLP on the whole group: a.T = tanh(Wa1 @ h.T + ba1)
                ap_ = psum_a.tile([P, GW], F32, tag="ap")
                for cc in range(nchk):
                    mv = ht[:, :, cc * CHUNK:(cc + 1) * CHUNK]
                    o = ap_[:, cc * CHUNK:(cc + 1) * CHUNK]
                    if fp8:
                        nc.tensor.matmul(o, wa1t_sb[:, :, :], mv,
                                         start=True, stop=True, perf_mode=DR)
                    else:
                        nc.tensor.matmul(o, wa1t_sb[:, 0, :], mv[:, 0, :],
                                         start=True, stop=False)
                        nc.tensor.matmul(o, wa1t_sb[:, 1, :], mv[:, 1, :],
                                         start=False, stop=True)
                at = apool.tile([P, GW], BF16, tag="at")
                nc.scalar.activation(at[:, 0:wg], ap_[:, 0:wg], AF.Tanh,
                                     bias=tanh_bias, scale=inv_scale)

                # scores broadcast across partitions via replicated Wa2
                sp = psum_s.tile([P, GW], F32, tag="sp")
                for cc in range(nchk):
                    nc.tensor.matmul(sp[:, cc * CHUNK:(cc + 1) * CHUNK],
                                     wa2r_sb[:], at[:, cc * CHUNK:(cc + 1) * CHUNK],
                                     start=True, stop=True)
                eb = epool.tile([P, GW], BF16, tag="eb")
                if gi >= n_groups - 2:
                    # final groups: per-chunk exp + stt so the tail drain
                    # pipelines at 512 granularity instead of serializing a
                    # full 1024-wide exp -> stt chain
                    for cc in range(nchk):
                        lo, hi = cc * CHUNK, (cc + 1) * CHUNK
                        nc.scalar.activation(eb[:, lo:hi], sp[:, lo:hi],
                                             AF.Exp,
                                             accum_out=zparts[:, slot:slot + 1])
                        pending.append((ht, eb, bpp, slot, lo, hi))
                        slot += 1
                else:
                    nc.scalar.activation(eb[:, 0:wg], sp[:, 0:wg], AF.Exp,
                                         accum_out=zparts[:, slot:slot + 1])
                    pending.append((ht, eb, bpp, slot, 0, wg))
                    slot += 1
                gi += 1
                goff += wg
                coff += wg
                if goff % bag == 0:
                    pend_fin = (b, zparts, bpp, slot)

        # flush the software pipeline
        for pend in pending:
            emit_stt(pend)
        emit_fin(pend_fin)

        # head: out = bag_emb @ Wh.T + bh  (weights from the packed consts)
        hdp = psum_s.tile([n_bags, HEAD_DIM], F32, tag="sp")
        nc.tensor.matmul(hdp[:], bagembT[:, 0, :], wpk_sb[:, 0:HEAD_DIM],
                         start=True, stop=False)
        nc.tensor.matmul(hdp[:], bagembT[:, 1, :], wpk_sb[:, HEAD_DIM:2 * HEAD_DIM],
                         start=False, stop=False)
        nc.tensor.matmul(hdp[:], wpk_sb[0:1, 6:6 + n_bags],
                         wpk_sb[0:1, 4:4 + HEAD_DIM],
                         start=False, stop=True)
        outt = const.tile([n_bags, HEAD_DIM], F32)
        nc.scalar.copy(outt[:], hdp[:])
        nc.sync.dma_start(out_ext.ap(), outt[:])

    nc.compile()
    return nc


def prep_weights(W1, b1, Wa1, ba1, Wa2, ba2, Wh, bh, n_bags, mode="fp8"):
    f32 = np.float32
    fp8 = mode == "fp8"
    wdt = NP_F8 if fp8 else NP_BF16
    ws = W_SCALE if fp8 else 1.0
    W1 = np.asarray(W1, f32)
    Wa1 = np.asarray(Wa1, f32)
    Wa2 = np.asarray(Wa2, f32)
    Wh = np.asarray(Wh, f32)
    # packed tail consts: [wht(2x2) | bh2(2) | ones(n_bags) | ba1c(1)]
    wpk = np.zeros((P, 7 + n_bags), f32)
    wpk[:, 0:4] = Wh.T.reshape(2, P, HEAD_DIM).transpose(1, 0, 2).reshape(P, 4)
    wpk[:, 4:6] = np.asarray(bh, f32).reshape(1, HEAD_DIM)
    wpk[:, 6:6 + n_bags] = 1.0
    wpk[:, 6 + n_bags] = np.asarray(ba1, f32).reshape(P)
    return {
        "w1t": (W1.T * ws).reshape(DC, P, FEAT_DIM).transpose(1, 0, 2).astype(wdt),
        "wa1t": (Wa1.T * ws).reshape(2, P, ATTN_DIM).transpose(1, 0, 2).astype(wdt),
        "wa2r": np.repeat(Wa2.reshape(P, 1), P, axis=1).astype(NP_BF16),
        "wpk": wpk.astype(NP_BF16),
        "b1c": np.ascontiguousarray(np.asarray(b1, f32).reshape(2, P).T),
        "ba1c": np.asarray(ba1, f32).reshape(P, 1).copy(),
    }


def prep_features(features, mode="fp8"):
    """Per-core [P, DC * NP_CORE] fp8/bf16 feature pack: concatenated DMA
    slabs, each slab [P, DC, w] c-major so every slab DMA is one contiguous
    per-partition run."""
    f32 = np.float32
    X = np.asarray(features, f32)
    xdt = NP_F8 if mode == "fp8" else NP_BF16
    outs = []
    for i in range(N_CORES):
        xc = X[i * NP_CORE:(i + 1) * NP_CORE].T.astype(xdt)   # [1024, 32768]
        xc = xc.reshape(DC, P, NP_CORE)
        parts = []
        off = 0
        for w in SLABS:
            parts.append(
                xc[:, :, off:off + w].transpose(1, 0, 2).reshape(P, DC * w))
            off += w
        outs.append(np.ascontiguousarray(np.concatenate(parts, axis=1)))
    return outs


_NC_CACHE = {}


def kernel(features, W1, b1, Wa1, ba1, Wa2, ba2, Wh, bh, bag_sizes):
    f32 = np.float32
    mode = "fp8"
    # the fp8 fast path folds b1 into a scalar activation bias, which is only
    # exact when b1 is all-zero (it is, for this model's inputs)
    if np.any(np.asarray(b1, f32) != 0.0):
        mode = "bf16"
    n_bags_core = NP_CORE // BAG

    shared = prep_weights(W1, b1, Wa1, ba1, Wa2, ba2, Wh, bh, n_bags_core, mode)
    xts = prep_features(features, mode)
    in_maps = [{**shared, "xt": xts[i]} for i in range(N_CORES)]

    if mode not in _NC_CACHE:
        _NC_CACHE[mode] = build_nc(mode=mode)
    nc = _NC_CACHE[mode]

    res = run_bass_kernel_spmd(nc, in_maps, core_ids=list(range(N_CORES)))
    out = np.concatenate(
        [np.asarray(res.results[i]["out"], f32) for i in range(N_CORES)], axis=0
    )
    return out


# revision 31
# speedup vs baseline: 1.0264x; 1.0264x over previous
"""AttentionMIL (segment softmax-attention reduce) Trainium2 kernel, 8 NeuronCores.

Model (per reference):
    h       = relu(features @ W1.T + b1)          # [N, 256]
    a       = tanh(h @ Wa1.T + ba1)               # [N, 128]
    scores  = a @ Wa2.T + ba2                     # [N]
    attn    = segment_softmax(scores, 32 bags of 8192)
    bag_emb = segment_sum(attn * h)               # [32, 256]
    out     = bag_emb @ Wh.T + bh                 # [32, 2]

Sharding: patches split 8 ways (32768 patches = 4 whole bags per core);
weights replicated; everything device-local, no collectives.

Host prep: features are transposed and quantized (fp8-e4m3 by default) into
per-DMA-slab contiguous blocks ([P, DC, w] c-major per slab) so every slab DMA
is one contiguous per-partition descriptor run. W1/Wa1 are scaled by 16 before
fp8 quantization (their 0.02-scale values would otherwise sit in the subnormal
range); the 1/16 is folded into the activations' free scale operand. Wa2 is
replicated into a [128, 128] stationary so the score matmul lands broadcast
across all 128 partitions — making softmax + weighted reduce pure free-axis
operations. Encoder and attention matmuls run fp8 DoubleRow.

The tensor engine is the bound resource (~2.35us per 512-patch chunk of
irreducible DR matmul). The attention-side elementwise work is batched over
1024-patch groups so ScalarE/VectorE instruction overheads stay below the PE
budget; groups taper to 512 at the start (compute begins on the first 0.5MB
slab) and at the end (halves the pipeline-drain tail). Weight/const loads ride
the idle GPSIMD SWDGE ring; startup slabs alternate the two HWDGE rings; a
burst of matmuls on zeros during the initial DMA window lifts the PE HAM clock
gate to 2.4 GHz before real work arrives. PSUM: 2x2 encoder banks + 2-bank
attention pre-act + 2-bank score buffer = all 8 banks.

Softmax max-subtraction is dropped: attn = e/z is exactly shift-invariant and
|scores| <= sum|Wa2| * 1 < 3, so exp cannot overflow. ba2 likewise cancels.
"""

import sys

if "/opt/trn_rl_repo" not in sys.path:
    sys.path.insert(0, "/opt/trn_rl_repo")

from contextlib import ExitStack

import ml_dtypes
import numpy as np

from concourse import bacc, mybir, tile
from concourse.bass_utils import run_bass_kernel_spmd

N_CORES = 8
N_PATCHES = 262144
INPUT_DIM = 1024
FEAT_DIM = 256
ATTN_DIM = 128
HEAD_DIM = 2
NP_CORE = N_PATCHES // N_CORES  # 32768
BAG = 8192

P = 128
DC = INPUT_DIM // P  # 8 contraction chunks of 128
CHUNK = 512          # patches per encoder tile (one PSUM bank at fp32)
GW = 2 * CHUNK       # max attention-group width
W_SCALE = 16.0       # host pre-scale on W1/Wa1 before fp8 quantization

# DMA slab schedule: small slabs to prime the pipeline, then 2MB slabs.
SLABS = [512, 512] + [1024] * 31
def _slab_groups(k):
    w = SLABS[k]
    return [w] if w < GW else [GW] * (w // GW)

ZMAX = 10            # z-partial slots per bag (tail groups split per chunk)

BF16 = mybir.dt.bfloat16
F32 = mybir.dt.float32
FP8 = mybir.dt.float8e4
AF = mybir.ActivationFunctionType
ALU = mybir.AluOpType
AX = mybir.AxisListType
DR = mybir.MatmulPerfMode.DoubleRow

NP_F8 = ml_dtypes.float8_e4m3
NP_BF16 = ml_dtypes.bfloat16


def build_nc(np_core=NP_CORE, bag=BAG, mode="fp8"):
    n_bags = np_core // bag       # bags per core = 4
    assert sum(SLABS) == np_core and np_core % bag == 0

    fp8 = mode == "fp8"
    XDT = FP8 if fp8 else BF16

    nc = bacc.Bacc()
    xt = nc.declare_dram_parameter("xt", [P, DC * np_core], XDT, isOutput=False)
    w1t = nc.declare_dram_parameter("w1t", [P, DC, FEAT_DIM], XDT, isOutput=False)
    wa1t = nc.declare_dram_parameter("wa1t", [P, 2, ATTN_DIM], XDT, isOutput=False)
    wa2r = nc.declare_dram_parameter("wa2r", [P, P], BF16, isOutput=False)
    # packed tail consts: [wht(4) | bh2(2) | ones(n_bags) | ba1c(1)] bf16
    wpk = nc.declare_dram_parameter("wpk", [P, 7 + n_bags], BF16, isOutput=False)
    b1c = nc.declare_dram_parameter("b1c", [P, 2], F32, isOutput=False)
    ba1c = nc.declare_dram_parameter("ba1c", [P, 1], F32, isOutput=False)
    out_ext = nc.declare_dram_parameter("out", [n_bags, HEAD_DIM], F32, isOutput=True)

    inv_scale = 1.0 / W_SCALE if fp8 else 1.0

    def slab_ap(off, w):
        return xt.ap()[:, off * DC:(off + w) * DC].rearrange(
            "p (c n) -> p c n", c=DC, n=w)

    with tile.TileContext(nc) as tc, ExitStack() as ctx:
        const = ctx.enter_context(tc.tile_pool(name="const", bufs=1))
        xpool = ctx.enter_context(tc.tile_pool(name="xpool", bufs=3))
        hpool = ctx.enter_context(tc.tile_pool(name="hpool", bufs=3))
        apool = ctx.enter_context(tc.tile_pool(name="apool", bufs=2))
        epool = ctx.enter_context(tc.tile_pool(name="epool", bufs=2))
        stpool = ctx.enter_context(tc.tile_pool(name="stpool", bufs=2))
        bpool = ctx.enter_context(tc.tile_pool(name="bpool", bufs=2))
        # PSUM (8 banks): hp 2 bufs x 2 banks + ap_g 2 banks + sp_g 2 banks
        psum_h = ctx.enter_context(tc.tile_pool(name="psum_h", bufs=2, space="PSUM"))
        psum_a = ctx.enter_context(tc.tile_pool(name="psum_a", bufs=1, space="PSUM"))
        psum_s = ctx.enter_context(tc.tile_pool(name="psum_s", bufs=1, space="PSUM"))

        # PE warm-up during the initial DMA window: back-to-back matmuls on
        # zeros lift the HAM clock gate to 2.4 GHz before the first real
        # encoder matmul issues, with no idle window that would re-throttle.
        dummy = const.tile([P, CHUNK], XDT)
        warm = psum_a.tile([P, GW], F32, tag="ap")
        nc.vector.memset(dummy[:], 0.0)
        for i in range(8):
            nc.tensor.matmul(warm[:, (i % 2) * CHUNK:(i % 2 + 1) * CHUNK],
                             dummy[:, 0:P], dummy[:],
                             start=True, stop=True)

        # All DMAs ride the sync-engine HWDGE ring in dependency order —
        # w1t before slab0 (both gate the first matmul), remaining weights
        # behind slab0. A second ring would let the deep slab queue starve
        # the weight transfers (measured: wa2r +8us late, PE stall, HAM
        # re-throttle).
        w1t_sb = const.tile([P, DC, FEAT_DIM], XDT)
        nc.sync.dma_start(w1t_sb[:], w1t.ap())
        wa1t_sb = const.tile([P, 2, ATTN_DIM], XDT)
        wa2r_sb = const.tile([P, P], BF16)
        wpk_sb = const.tile([P, 7 + n_bags], BF16)
        b1c_sb = const.tile([P, 2], F32)
        ba1c_sb = const.tile([P, 1], F32)

        def load_late_consts():
            nc.sync.dma_start(wa1t_sb[:], wa1t.ap())
            nc.sync.dma_start(wa2r_sb[:], wa2r.ap())
            nc.sync.dma_start(wpk_sb[:], wpk.ap())
            if not fp8:
                nc.sync.dma_start(b1c_sb[:], b1c.ap())
                nc.sync.dma_start(ba1c_sb[:], ba1c.ap())

        tanh_bias = wpk_sb[:, 6 + n_bags:7 + n_bags] if fp8 else ba1c_sb[:]
        bagembT = const.tile([P, 2, n_bags], BF16)  # normalized bag embeddings

        def emit_stt(pend):
            # weighted partial reduce: bpp[f, fh, slot] = sum_p h.T*e, fused
            # into the multiply via the accumulate output. Deferred by one
            # group (software pipelining) so these never block the next
            # group's relu in the VectorE FIFO behind a pending exp.
            ht_, eb_, bpp_, pslot, lo, hi = pend
            st = stpool.tile([P, 2, GW], BF16, tag="st")
            nc.vector.scalar_tensor_tensor(
                st[:, 0, lo:hi], ht_[:, 0, lo:hi], 1.0, eb_[:, lo:hi],
                op0=ALU.mult, op1=ALU.mult,
                accum_out=bpp_[:, 0, pslot:pslot + 1])
            nc.vector.scalar_tensor_tensor(
                st[:, 1, lo:hi], ht_[:, 1, lo:hi], 1.0, eb_[:, lo:hi],
                op0=ALU.mult, op1=ALU.mult,
                accum_out=bpp_[:, 1, pslot:pslot + 1])

        def emit_fin(fin):
            # finish bag b: z = sum(zparts); bagembT[:, :, b] = sum/z
            b_, zp_, bpp_, ns = fin
            zs = bpool.tile([P, 1], F32, tag="zs")
            nc.vector.tensor_reduce(zs[:], zp_[:, 0:ns], axis=AX.X, op=ALU.add)
            rz = bpool.tile([P, 1], F32, tag="rz")
            nc.vector.reciprocal(rz[:], zs[:])
            s0 = bpool.tile([P, 1], F32, tag="s0")
            nc.vector.tensor_reduce(s0[:], bpp_[:, 0, 0:ns], axis=AX.X, op=ALU.add)
            s1 = bpool.tile([P, 1], F32, tag="s1")
            nc.vector.tensor_reduce(s1[:], bpp_[:, 1, 0:ns], axis=AX.X, op=ALU.add)
            nc.vector.tensor_mul(bagembT[:, 0, b_:b_ + 1], s0[:], rz[:])
            nc.vector.tensor_mul(bagembT[:, 1, b_:b_ + 1], s1[:], rz[:])

        zparts = bpp = None
        pending = []     # (ht, eb, bpp, slot, lo, hi) awaiting their stt
        pend_fin = None  # (b, zparts, bpp, nslots) awaiting bag finalize
        slot = 0
        goff = 0  # global patch offset
        off = 0
        n_groups = sum(len(_slab_groups(k)) for k in range(len(SLABS)))
        gi = 0
        for k, w in enumerate(SLABS):
            xsb = xpool.tile([P, DC, w], XDT, tag=f"x{w}")
            nc.sync.dma_start(xsb[:], slab_ap(off, w))
            if k == 0:
                load_late_consts()
            off += w
            coff = 0  # patch offset within slab
            for wg in _slab_groups(k):
                b = goff // bag
                if goff % bag == 0:
                    slot = 0
                    zparts = bpool.tile([P, ZMAX], F32, tag="zparts")
                    bpp = bpool.tile([P, 2, ZMAX], F32, tag="bpp")
                nchk = wg // CHUNK

                # encoder for the group's 512-patch chunks
                ht = hpool.tile([P, 2, GW], XDT, tag="ht")
                for cc in range(nchk):
                    cs = coff + cc * CHUNK
                    rhs = xsb[:, :, cs:cs + CHUNK]
                    hp = psum_h.tile([P, 2, CHUNK], F32, tag="hp")
                    for fh in range(2):
                        if fp8:
                            for d in range(DC // 2):
                                nc.tensor.matmul(
                                    hp[:, fh, :],
                                    w1t_sb[:, 2 * d:2 * d + 2, fh * P:(fh + 1) * P],
                                    rhs[:, 2 * d:2 * d + 2, :],
                                    start=(d == 0), stop=(d == DC // 2 - 1),
                                    perf_mode=DR,
                                )
                        else:
                            for d in range(DC):
                                nc.tensor.matmul(
                                    hp[:, fh, :],
                                    w1t_sb[:, d, fh * P:(fh + 1) * P],
                                    rhs[:, d, :],
                                    start=(d == 0), stop=(d == DC - 1),
                                )
                    ho = ht[:, :, cc * CHUNK:(cc + 1) * CHUNK]
                    if fp8:
                        # b1 == 0 in this model (checked in kernel()); split
                        # the two feature halves across ScalarE and VectorE
                        nc.scalar.activation(ho[:, 0, :], hp[:, 0, :], AF.Relu,
                                             bias=0.0, scale=inv_scale)
                        nc.vector.tensor_scalar(ho[:, 1, :], hp[:, 1, :],
                                                inv_scale, 0.0,
                                                op0=ALU.mult, op1=ALU.max)
                    else:
                        nc.scalar.activation(ho[:, 0, :], hp[:, 0, :], AF.Relu,
                                             bias=b1c_sb[:, 0:1])
                        nc.scalar.activation(ho[:, 1, :], hp[:, 1, :], AF.Relu,
                                             bias=b1c_sb[:, 1:2])

                # previous group's weighted reduce + any completed bag
                for pend in pending:
                    emit_stt(pend)
                pending = []
                if pend_fin is not None:
                    emit_fin(pend_fin)
                    pend_fin = None

                # attention MLP on the whole group: a.T = tanh(Wa1 @ h.T + ba1)
                ap_ = psum_a.tile([P, GW], F32, tag="ap")
                for cc in range(nchk):
                    mv = ht[:, :, cc * CHUNK:(cc + 1) * CHUNK]
                    o = ap_[:, cc * CHUNK:(cc + 1) * CHUNK]
                    if fp8:
                        nc.tensor.matmul(o, wa1t_sb[:, :, :], mv,
                                         start=True, stop=True, perf_mode=DR)
                    else:
                        nc.tensor.matmul(o, wa1t_sb[:, 0, :], mv[:, 0, :],
                                         start=True, stop=False)
                        nc.tensor.matmul(o, wa1t_sb[:, 1, :], mv[:, 1, :],
                                         start=False, stop=True)
                at = apool.tile([P, GW], BF16, tag="at")
                nc.scalar.activation(at[:, 0:wg], ap_[:, 0:wg], AF.Tanh,
                                     bias=tanh_bias, scale=inv_scale)

                # scores broadcast across partitions via replicated Wa2
                sp = psum_s.tile([P, GW], F32, tag="sp")
                for cc in range(nchk):
                    nc.tensor.matmul(sp[:, cc * CHUNK:(cc + 1) * CHUNK],
                                     wa2r_sb[:], at[:, cc * CHUNK:(cc + 1) * CHUNK],
                                     start=True, stop=True)
                eb = epool.tile([P, GW], BF16, tag="eb")
                if gi >= n_groups - 2:
                    # final groups: per-chunk exp + stt so the tail drain
                    # pipelines at 512 granularity instead of serializing a
                    # full 1024-wide exp -> stt chain
                    for cc in range(nchk):
                        lo, hi = cc * CHUNK, (cc + 1) * CHUNK
                        nc.scalar.activation(eb[:, lo:hi], sp[:, lo:hi],
                                             AF.Exp,
                                             accum_out=zparts[:, slot:slot + 1])
                        pending.append((ht, eb, bpp, slot, lo, hi))
                        slot += 1
                else:
                    nc.scalar.activation(eb[:, 0:wg], sp[:, 0:wg], AF.Exp,
                                         accum_out=zparts[:, slot:slot + 1])
                    pending.append((ht, eb, bpp, slot, 0, wg))
                    slot += 1
                gi += 1
                goff += wg
                coff += wg
                if goff % bag == 0:
                    pend_fin = (b, zparts, bpp, slot)

        # flush the software pipeline
        for pend in pending:
            emit_stt(pend)
        emit_fin(pend_fin)

        # head: out = bag_emb @ Wh.T + bh  (weights from the packed consts)
        hdp = psum_s.tile([n_bags, HEAD_DIM], F32, tag="sp")
        nc.tensor.matmul(hdp[:], bagembT[:, 0, :], wpk_sb[:, 0:HEAD_DIM],
                         start=True, stop=False)
        nc.tensor.matmul(hdp[:], bagembT[:, 1, :], wpk_sb[:, HEAD_DIM:2 * HEAD_DIM],
                         start=False, stop=False)
        nc.tensor.matmul(hdp[:], wpk_sb[0:1, 6:6 + n_bags],
                         wpk_sb[0:1, 4:4 + HEAD_DIM],
                         start=False, stop=True)
        outt = const.tile([n_bags, HEAD_DIM], F32)
        nc.scalar.copy(outt[:], hdp[:])
        nc.sync.dma_start(out_ext.ap(), outt[:])

    nc.compile()
    return nc


def prep_weights(W1, b1, Wa1, ba1, Wa2, ba2, Wh, bh, n_bags, mode="fp8"):
    f32 = np.float32
    fp8 = mode == "fp8"
    wdt = NP_F8 if fp8 else NP_BF16
    ws = W_SCALE if fp8 else 1.0
    W1 = np.asarray(W1, f32)
    Wa1 = np.asarray(Wa1, f32)
    Wa2 = np.asarray(Wa2, f32)
    Wh = np.asarray(Wh, f32)
    # packed tail consts: [wht(2x2) | bh2(2) | ones(n_bags) | ba1c(1)]
    wpk = np.zeros((P, 7 + n_bags), f32)
    wpk[:, 0:4] = Wh.T.reshape(2, P, HEAD_DIM).transpose(1, 0, 2).reshape(P, 4)
    wpk[:, 4:6] = np.asarray(bh, f32).reshape(1, HEAD_DIM)
    wpk[:, 6:6 + n_bags] = 1.0
    wpk[:, 6 + n_bags] = np.asarray(ba1, f32).reshape(P)
    return {
        "w1t": (W1.T * ws).reshape(DC, P, FEAT_DIM).transpose(1, 0, 2).astype(wdt),
        "wa1t": (Wa1.T * ws).reshape(2, P, ATTN_DIM).transpose(1, 0, 2).astype(wdt),
        "wa2r": np.repeat(Wa2.reshape(P, 1), P, axis=1).astype(NP_BF16),
        "wpk": wpk.astype(NP_BF16),
        "b1c": np.ascontiguousarray(np.asarray(b1, f32).reshape(2, P).T),
        "ba1c": np.asarray(ba1, f32).reshape(P, 1).copy(),
    }


def prep_features(features, mode="fp8"):
    """Per-core [P, DC * NP_CORE] fp8/bf16 feature pack: concatenated DMA
    slabs, each slab [P, DC, w] c-major so every slab DMA is one contiguous
    per-partition run."""
    f32 = np.float32
    X = np.asarray(features, f32)
    xdt = NP_F8 if mode == "fp8" else NP_BF16
    outs = []
    for i in range(N_CORES):
        xc = X[i * NP_CORE:(i + 1) * NP_CORE].T.astype(xdt)   # [1024, 32768]
        xc = xc.reshape(DC, P, NP_CORE)
        parts = []
        off = 0
        for w in SLABS:
            parts.append(
                xc[:, :, off:off + w].transpose(1, 0, 2).reshape(P, DC * w))
            off += w
        outs.append(np.ascontiguousarray(np.concatenate(parts, axis=1)))
    return outs


_NC_CACHE = {}


def kernel(features, W1, b1, Wa1, ba1, Wa2, ba2, Wh, bh, bag_sizes):
    f32 = np.float32
    mode = "fp8"
    # the fp8 fast path folds b1 into a scalar activation bias, which is only
    # exact when b1 is all-zero (it is, for this model's inputs)
    if np.any(np.asarray(b1, f32) != 0.0):
        mode = "bf16"
    n_bags_core = NP_CORE // BAG

    shared = prep_weights(W1, b1, Wa1, ba1, Wa2, ba2, Wh, bh, n_bags_core, mode)
    xts = prep_features(features, mode)
    in_maps = [{**shared, "xt": xts[i]} for i in range(N_CORES)]

    if mode not in _NC_CACHE:
        _NC_CACHE[mode] = build_nc(mode=mode)
    nc = _NC_CACHE[mode]

    res = run_bass_kernel_spmd(nc, in_maps, core_ids=list(range(N_CORES)))
    out = np.concatenate(
        [np.asarray(res.results[i]["out"], f32) for i in range(N_CORES)], axis=0
    )
    return out


# revision 33
# speedup vs baseline: 1.0588x; 1.0316x over previous
"""AttentionMIL (segment softmax-attention reduce) Trainium2 kernel, 8 NeuronCores.

Model (per reference):
    h       = relu(features @ W1.T + b1)          # [N, 256]
    a       = tanh(h @ Wa1.T + ba1)               # [N, 128]
    scores  = a @ Wa2.T + ba2                     # [N]
    attn    = segment_softmax(scores, 32 bags of 8192)
    bag_emb = segment_sum(attn * h)               # [32, 256]
    out     = bag_emb @ Wh.T + bh                 # [32, 2]

Sharding: patches split 8 ways (32768 patches = 4 whole bags per core);
weights replicated; everything device-local, no collectives.

Host prep: features are transposed and quantized (fp8-e4m3 by default) into
per-DMA-slab contiguous blocks ([P, DC, w] c-major per slab) so every slab DMA
is one contiguous per-partition descriptor run. W1/Wa1 are scaled by 16 before
fp8 quantization (their 0.02-scale values would otherwise sit in the subnormal
range); the 1/16 is folded into the activations' free scale operand. Wa2 is
replicated into a [128, 128] stationary so the score matmul lands broadcast
across all 128 partitions — making softmax + weighted reduce pure free-axis
operations. Encoder and attention matmuls run fp8 DoubleRow.

The tensor engine is the bound resource (~2.35us per 512-patch chunk of
irreducible DR matmul). The attention-side elementwise work is batched over
1024-patch groups so ScalarE/VectorE instruction overheads stay below the PE
budget; groups taper to 512 at the start (compute begins on the first 0.5MB
slab) and at the end (halves the pipeline-drain tail). Weight/const loads ride
the idle GPSIMD SWDGE ring; startup slabs alternate the two HWDGE rings; a
burst of matmuls on zeros during the initial DMA window lifts the PE HAM clock
gate to 2.4 GHz before real work arrives. PSUM: 2x2 encoder banks + 2-bank
attention pre-act + 2-bank score buffer = all 8 banks.

Softmax max-subtraction is dropped: attn = e/z is exactly shift-invariant and
|scores| <= sum|Wa2| * 1 < 3, so exp cannot overflow. ba2 likewise cancels.
"""

import sys

if "/opt/trn_rl_repo" not in sys.path:
    sys.path.insert(0, "/opt/trn_rl_repo")

from contextlib import ExitStack

import ml_dtypes
import numpy as np

from concourse import bacc, mybir, tile
from concourse.bass_utils import run_bass_kernel_spmd

N_CORES = 8
N_PATCHES = 262144
INPUT_DIM = 1024
FEAT_DIM = 256
ATTN_DIM = 128
HEAD_DIM = 2
NP_CORE = N_PATCHES // N_CORES  # 32768
BAG = 8192

P = 128
DC = INPUT_DIM // P  # 8 contraction chunks of 128
CHUNK = 512          # patches per encoder tile (one PSUM bank at fp32)
GW = 2 * CHUNK       # max attention-group width
W_SCALE = 16.0       # host pre-scale on W1/Wa1 before fp8 quantization

# DMA slab schedule: small slabs to prime the pipeline, then 2MB slabs.
SLABS = [512, 512] + [1024] * 31
def _slab_groups(k):
    w = SLABS[k]
    return [w] if w < GW else [GW] * (w // GW)

ZMAX = 10            # z-partial slots per bag (tail groups split per chunk)

BF16 = mybir.dt.bfloat16
F32 = mybir.dt.float32
FP8 = mybir.dt.float8e4
AF = mybir.ActivationFunctionType
ALU = mybir.AluOpType
AX = mybir.AxisListType
DR = mybir.MatmulPerfMode.DoubleRow

NP_F8 = ml_dtypes.float8_e4m3
NP_BF16 = ml_dtypes.bfloat16


def build_nc(np_core=NP_CORE, bag=BAG, mode="fp8"):
    n_bags = np_core // bag       # bags per core = 4
    assert sum(SLABS) == np_core and np_core % bag == 0

    fp8 = mode == "fp8"
    XDT = FP8 if fp8 else BF16

    nc = bacc.Bacc()
    xt = nc.declare_dram_parameter("xt", [P, DC * np_core], XDT, isOutput=False)
    w1t = nc.declare_dram_parameter("w1t", [P, DC, FEAT_DIM], XDT, isOutput=False)
    wa1t = nc.declare_dram_parameter("wa1t", [P, 2, ATTN_DIM], XDT, isOutput=False)
    wa2r = nc.declare_dram_parameter("wa2r", [P, P], BF16, isOutput=False)
    # packed tail consts: [wht(4) | bh2(2) | ones(n_bags) | ba1c(1)] bf16
    wpk = nc.declare_dram_parameter("wpk", [P, 7 + n_bags], BF16, isOutput=False)
    b1c = nc.declare_dram_parameter("b1c", [P, 2], F32, isOutput=False)
    ba1c = nc.declare_dram_parameter("ba1c", [P, 1], F32, isOutput=False)
    out_ext = nc.declare_dram_parameter("out", [n_bags, HEAD_DIM], F32, isOutput=True)

    inv_scale = 1.0 / W_SCALE if fp8 else 1.0

    def slab_ap(off, w):
        return xt.ap()[:, off * DC:(off + w) * DC].rearrange(
            "p (c n) -> p c n", c=DC, n=w)

    with tile.TileContext(nc) as tc, ExitStack() as ctx:
        const = ctx.enter_context(tc.tile_pool(name="const", bufs=1))
        xpool = ctx.enter_context(tc.tile_pool(name="xpool", bufs=3))
        hpool = ctx.enter_context(tc.tile_pool(name="hpool", bufs=3))
        apool = ctx.enter_context(tc.tile_pool(name="apool", bufs=2))
        epool = ctx.enter_context(tc.tile_pool(name="epool", bufs=2))
        stpool = ctx.enter_context(tc.tile_pool(name="stpool", bufs=2))
        bpool = ctx.enter_context(tc.tile_pool(name="bpool", bufs=2))
        # PSUM (8 banks): hp 2 bufs x 2 banks + ap_g 2 banks + sp_g 2 banks
        psum_h = ctx.enter_context(tc.tile_pool(name="psum_h", bufs=2, space="PSUM"))
        psum_a = ctx.enter_context(tc.tile_pool(name="psum_a", bufs=1, space="PSUM"))
        psum_s = ctx.enter_context(tc.tile_pool(name="psum_s", bufs=1, space="PSUM"))

        # PE warm-up during the initial DMA window: back-to-back matmuls on
        # zeros lift the HAM clock gate to 2.4 GHz before the first real
        # encoder matmul issues, with no idle window that would re-throttle.
        dummy = const.tile([P, CHUNK], XDT)
        warm = psum_a.tile([P, GW], F32, tag="ap")
        nc.vector.memset(dummy[:], 0.0)
        for i in range(8):
            nc.tensor.matmul(warm[:, (i % 2) * CHUNK:(i % 2 + 1) * CHUNK],
                             dummy[:, 0:P], dummy[:],
                             start=True, stop=True)

        # All DMAs ride the sync-engine HWDGE ring in dependency order —
        # w1t before slab0 (both gate the first matmul), remaining weights
        # behind slab0. A second ring would let the deep slab queue starve
        # the weight transfers (measured: wa2r +8us late, PE stall, HAM
        # re-throttle).
        w1t_sb = const.tile([P, DC, FEAT_DIM], XDT)
        nc.sync.dma_start(w1t_sb[:], w1t.ap())
        wa1t_sb = const.tile([P, 2, ATTN_DIM], XDT)
        wa2r_sb = const.tile([P, P], BF16)
        wpk_sb = const.tile([P, 7 + n_bags], BF16)
        b1c_sb = const.tile([P, 2], F32)
        ba1c_sb = const.tile([P, 1], F32)

        def load_late_consts():
            nc.sync.dma_start(wa1t_sb[:], wa1t.ap())
            nc.sync.dma_start(wa2r_sb[:], wa2r.ap())
            nc.sync.dma_start(wpk_sb[:], wpk.ap())
            if not fp8:
                nc.sync.dma_start(b1c_sb[:], b1c.ap())
                nc.sync.dma_start(ba1c_sb[:], ba1c.ap())

        tanh_bias = wpk_sb[:, 6 + n_bags:7 + n_bags] if fp8 else ba1c_sb[:]
        bagembT = const.tile([P, 2, n_bags], BF16)  # normalized bag embeddings

        def emit_stt(pend):
            # weighted partial reduce: bpp[f, fh, slot] = sum_p h.T*e, fused
            # into the multiply via the accumulate output. Deferred by one
            # group (software pipelining) so these never block the next
            # group's relu in the VectorE FIFO behind a pending exp.
            ht_, eb_, bpp_, pslot, lo, hi, _fin = pend
            st = stpool.tile([P, 2, GW], BF16, tag="st")
            nc.vector.scalar_tensor_tensor(
                st[:, 0, lo:hi], ht_[:, 0, lo:hi], 1.0, eb_[:, lo:hi],
                op0=ALU.mult, op1=ALU.mult,
                accum_out=bpp_[:, 0, pslot:pslot + 1])
            nc.vector.scalar_tensor_tensor(
                st[:, 1, lo:hi], ht_[:, 1, lo:hi], 1.0, eb_[:, lo:hi],
                op0=ALU.mult, op1=ALU.mult,
                accum_out=bpp_[:, 1, pslot:pslot + 1])

        def emit_fin(fin):
            # finish bag b: z = sum(zparts); bagembT[:, :, b] = sum/z
            b_, zp_, bpp_, ns = fin
            zs = bpool.tile([P, 1], F32, tag="zs")
            nc.vector.tensor_reduce(zs[:], zp_[:, 0:ns], axis=AX.X, op=ALU.add)
            rz = bpool.tile([P, 1], F32, tag="rz")
            nc.vector.reciprocal(rz[:], zs[:])
            s0 = bpool.tile([P, 1], F32, tag="s0")
            nc.vector.tensor_reduce(s0[:], bpp_[:, 0, 0:ns], axis=AX.X, op=ALU.add)
            s1 = bpool.tile([P, 1], F32, tag="s1")
            nc.vector.tensor_reduce(s1[:], bpp_[:, 1, 0:ns], axis=AX.X, op=ALU.add)
            nc.vector.tensor_mul(bagembT[:, 0, b_:b_ + 1], s0[:], rz[:])
            nc.vector.tensor_mul(bagembT[:, 1, b_:b_ + 1], s1[:], rz[:])

        # The whole attention stage (wa1, tanh, score, exp) is software-
        # pipelined one group behind the encoder: every PE op in the steady
        # state depends only on results a full group old, so no PE<->ACT
        # round trip sits on the critical path. The weighted reduce (stt)
        # trails one further group.
        def emit_att_a(a):
            # wa1 matmuls + tanh for the previous group (ACT gets the tanh
            # ahead of this group's relus in its FIFO)
            ht_, zp_, bpp_, slots_, wg_, fin_ = a
            nchk_ = wg_ // CHUNK
            ap_ = psum_a.tile([P, GW], F32, tag="ap")
            for cc in range(nchk_):
                mv = ht_[:, :, cc * CHUNK:(cc + 1) * CHUNK]
                o = ap_[:, cc * CHUNK:(cc + 1) * CHUNK]
                if fp8:
                    nc.tensor.matmul(o, wa1t_sb[:, :, :], mv,
                                     start=True, stop=True, perf_mode=DR)
                else:
                    nc.tensor.matmul(o, wa1t_sb[:, 0, :], mv[:, 0, :],
                                     start=True, stop=False)
                    nc.tensor.matmul(o, wa1t_sb[:, 1, :], mv[:, 1, :],
                                     start=False, stop=True)
            at = apool.tile([P, GW], BF16, tag="at")
            nc.scalar.activation(at[:, 0:wg_], ap_[:, 0:wg_], AF.Tanh,
                                 bias=tanh_bias, scale=inv_scale)
            return at

        def emit_att_b(a, at):
            # score matmuls + exp for the previous group; queues its stt
            ht_, zp_, bpp_, slots_, wg_, fin_ = a
            nchk_ = wg_ // CHUNK
            sp = psum_s.tile([P, GW], F32, tag="sp")
            for cc in range(nchk_):
                nc.tensor.matmul(sp[:, cc * CHUNK:(cc + 1) * CHUNK],
                                 wa2r_sb[:], at[:, cc * CHUNK:(cc + 1) * CHUNK],
                                 start=True, stop=True)
            eb = epool.tile([P, GW], BF16, tag="eb")
            out = []
            if len(slots_) > 1:
                # final groups: per-chunk exp + stt so the tail drain
                # pipelines at 512 granularity
                for cc in range(nchk_):
                    lo, hi = cc * CHUNK, (cc + 1) * CHUNK
                    nc.scalar.activation(eb[:, lo:hi], sp[:, lo:hi], AF.Exp,
                                         accum_out=zp_[:, slots_[cc]:slots_[cc] + 1])
                    out.append([ht_, eb, bpp_, slots_[cc], lo, hi, None])
            else:
                nc.scalar.activation(eb[:, 0:wg_], sp[:, 0:wg_], AF.Exp,
                                     accum_out=zp_[:, slots_[0]:slots_[0] + 1])
                out.append([ht_, eb, bpp_, slots_[0], 0, wg_, None])
            out[-1][6] = fin_
            return out

        zparts = bpp = None
        att = None       # previous group awaiting its attention stage
        stt_ready = []   # stt descriptors whose eb landed last block
        slot = 0
        goff = 0  # global patch offset
        off = 0
        n_groups = sum(len(_slab_groups(k)) for k in range(len(SLABS)))
        gi = 0
        for k, w in enumerate(SLABS):
            xsb = xpool.tile([P, DC, w], XDT, tag=f"x{w}")
            nc.sync.dma_start(xsb[:], slab_ap(off, w))
            if k == 0:
                load_late_consts()
            off += w
            coff = 0  # patch offset within slab
            for wg in _slab_groups(k):
                b = goff // bag
                if goff % bag == 0:
                    slot = 0
                    zparts = bpool.tile([P, ZMAX], F32, tag="zparts")
                    bpp = bpool.tile([P, 2, ZMAX], F32, tag="bpp")
                nchk = wg // CHUNK
                split = gi >= n_groups - 2
                slots_g = list(range(slot, slot + (nchk if split else 1)))
                slot += len(slots_g)
                fin = None
                if (goff + wg) % bag == 0:
                    fin = (b, zparts, bpp, slot)

                # encoder for the group's 512-patch chunks, with the previous
                # group's attention stage threaded between them
                ht = hpool.tile([P, 2, GW], XDT, tag="ht")
                at_prev = None
                for cc in range(nchk):
                    cs = coff + cc * CHUNK
                    rhs = xsb[:, :, cs:cs + CHUNK]
                    hp = psum_h.tile([P, 2, CHUNK], F32, tag="hp")
                    for fh in range(2):
                        if fp8:
                            for d in range(DC // 2):
                                nc.tensor.matmul(
                                    hp[:, fh, :],
                                    w1t_sb[:, 2 * d:2 * d + 2, fh * P:(fh + 1) * P],
                                    rhs[:, 2 * d:2 * d + 2, :],
                                    start=(d == 0), stop=(d == DC // 2 - 1),
                                    perf_mode=DR,
                                )
                        else:
                            for d in range(DC):
                                nc.tensor.matmul(
                                    hp[:, fh, :],
                                    w1t_sb[:, d, fh * P:(fh + 1) * P],
                                    rhs[:, d, :],
                                    start=(d == 0), stop=(d == DC - 1),
                                )
                    if cc == 0 and att is not None:
                        at_prev = emit_att_a(att)
                    ho = ht[:, :, cc * CHUNK:(cc + 1) * CHUNK]
                    if fp8:
                        # b1 == 0 in this model (checked in kernel()); split
                        # the two feature halves across ScalarE and VectorE
                        nc.scalar.activation(ho[:, 0, :], hp[:, 0, :], AF.Relu,
                                             bias=0.0, scale=inv_scale)
                        nc.vector.tensor_scalar(ho[:, 1, :], hp[:, 1, :],
                                                inv_scale, 0.0,
                                                op0=ALU.mult, op1=ALU.max)
                    else:
                        nc.scalar.activation(ho[:, 0, :], hp[:, 0, :], AF.Relu,
                                             bias=b1c_sb[:, 0:1])
                        nc.scalar.activation(ho[:, 1, :], hp[:, 1, :], AF.Relu,
                                             bias=b1c_sb[:, 1:2])

                stt_new = []
                if att is not None:
                    stt_new = emit_att_b(att, at_prev)
                # stt + bag finalize for descriptors queued one block earlier
                for pend in stt_ready:
                    emit_stt(pend)
                    if pend[6] is not None:
                        emit_fin(pend[6])
                stt_ready = stt_new

                att = (ht, zparts, bpp, slots_g, wg, fin)
                gi += 1
                goff += wg
                coff += wg

        # drain the software pipeline: last group's attention, then all
        # outstanding weighted reduces and bag finalizes
        at_prev = emit_att_a(att)
        stt_new = emit_att_b(att, at_prev)
        for pend in stt_ready + stt_new:
            emit_stt(pend)
            if pend[6] is not None:
                emit_fin(pend[6])

        # head: out = bag_emb @ Wh.T + bh  (weights from the packed consts)
        hdp = psum_s.tile([n_bags, HEAD_DIM], F32, tag="sp")
        nc.tensor.matmul(hdp[:], bagembT[:, 0, :], wpk_sb[:, 0:HEAD_DIM],
                         start=True, stop=False)
        nc.tensor.matmul(hdp[:], bagembT[:, 1, :], wpk_sb[:, HEAD_DIM:2 * HEAD_DIM],
                         start=False, stop=False)
        nc.tensor.matmul(hdp[:], wpk_sb[0:1, 6:6 + n_bags],
                         wpk_sb[0:1, 4:4 + HEAD_DIM],
                         start=False, stop=True)
        outt = const.tile([n_bags, HEAD_DIM], F32)
        nc.scalar.copy(outt[:], hdp[:])
        nc.sync.dma_start(out_ext.ap(), outt[:])

    nc.compile()
    return nc


def prep_weights(W1, b1, Wa1, ba1, Wa2, ba2, Wh, bh, n_bags, mode="fp8"):
    f32 = np.float32
    fp8 = mode == "fp8"
    wdt = NP_F8 if fp8 else NP_BF16
    ws = W_SCALE if fp8 else 1.0
    W1 = np.asarray(W1, f32)
    Wa1 = np.asarray(Wa1, f32)
    Wa2 = np.asarray(Wa2, f32)
    Wh = np.asarray(Wh, f32)
    # packed tail consts: [wht(2x2) | bh2(2) | ones(n_bags) | ba1c(1)]
    wpk = np.zeros((P, 7 + n_bags), f32)
    wpk[:, 0:4] = Wh.T.reshape(2, P, HEAD_DIM).transpose(1, 0, 2).reshape(P, 4)
    wpk[:, 4:6] = np.asarray(bh, f32).reshape(1, HEAD_DIM)
    wpk[:, 6:6 + n_bags] = 1.0
    wpk[:, 6 + n_bags] = np.asarray(ba1, f32).reshape(P)
    return {
        "w1t": (W1.T * ws).reshape(DC, P, FEAT_DIM).transpose(1, 0, 2).astype(wdt),
        "wa1t": (Wa1.T * ws).reshape(2, P, ATTN_DIM).transpose(1, 0, 2).astype(wdt),
        "wa2r": np.repeat(Wa2.reshape(P, 1), P, axis=1).astype(NP_BF16),
        "wpk": wpk.astype(NP_BF16),
        "b1c": np.ascontiguousarray(np.asarray(b1, f32).reshape(2, P).T),
        "ba1c": np.asarray(ba1, f32).reshape(P, 1).copy(),
    }


def prep_features(features, mode="fp8"):
    """Per-core [P, DC * NP_CORE] fp8/bf16 feature pack: concatenated DMA
    slabs, each slab [P, DC, w] c-major so every slab DMA is one contiguous
    per-partition run."""
    f32 = np.float32
    X = np.asarray(features, f32)
    xdt = NP_F8 if mode == "fp8" else NP_BF16
    outs = []
    for i in range(N_CORES):
        xc = X[i * NP_CORE:(i + 1) * NP_CORE].T.astype(xdt)   # [1024, 32768]
        xc = xc.reshape(DC, P, NP_CORE)
        parts = []
        off = 0
        for w in SLABS:
            parts.append(
                xc[:, :, off:off + w].transpose(1, 0, 2).reshape(P, DC * w))
            off += w
        outs.append(np.ascontiguousarray(np.concatenate(parts, axis=1)))
    return outs


_NC_CACHE = {}


def kernel(features, W1, b1, Wa1, ba1, Wa2, ba2, Wh, bh, bag_sizes):
    f32 = np.float32
    mode = "fp8"
    # the fp8 fast path folds b1 into a scalar activation bias, which is only
    # exact when b1 is all-zero (it is, for this model's inputs)
    if np.any(np.asarray(b1, f32) != 0.0):
        mode = "bf16"
    n_bags_core = NP_CORE // BAG

    shared = prep_weights(W1, b1, Wa1, ba1, Wa2, ba2, Wh, bh, n_bags_core, mode)
    xts = prep_features(features, mode)
    in_maps = [{**shared, "xt": xts[i]} for i in range(N_CORES)]

    if mode not in _NC_CACHE:
        _NC_CACHE[mode] = build_nc(mode=mode)
    nc = _NC_CACHE[mode]

    res = run_bass_kernel_spmd(nc, in_maps, core_ids=list(range(N_CORES)))
    out = np.concatenate(
        [np.asarray(res.results[i]["out"], f32) for i in range(N_CORES)], axis=0
    )
    return out
